# revision 5
# baseline (speedup 1.0000x reference)
"""DualPathAttention Trainium2 kernel.

Computes, for each batch row of x [S=512, D=512]:
  global branch: 8-head full self-attention + out-proj
  local branch:  overlapping-window (W=10, stride 5) 4-head attention,
                 scatter-added, + out-proj (folded through the scatter)
  fusion:        relu(concat(global, local) @ fw.T)

Strategy: data-parallel over batch B=32 across 8 NeuronCores (4 batches
per core).  All dense matmuls run in bfloat16 (fp32 PSUM accumulate);
softmax denominators and reciprocals stay fp32 for scale accuracy.

Local attention is decomposed into two block-diagonal phases:
  phase 0 = even windows (starts 0,10,...,510) — aligned 10-token blocks
  phase 1 = odd windows (starts 5,15,...,505) — blocks offset by 5
Each token belongs to exactly one window per phase; the reference's
scatter-add equals (phase0_out + phase1_out), accumulated in PSUM.
Queries are processed in groups of 110 tokens; per-window softmax uses a
block-diagonal mask, exp without max subtraction (scores are ~±1.5), and
denominators via an all-ones stationary matmul (replicated across
partitions) + DVE reciprocal.
"""
import ml_dtypes
import numpy as np

B, S, D = 32, 512, 512
GH, LH = 8, 4
GDH, LDH = D // GH, D // LH          # 64, 128
W, STRIDE = 10, 5
NCORES = 8
BPC = B // NCORES                     # batches per core
GRP = 110                             # local query group size
GROUPS = [(g, min(g + GRP, S)) for g in range(0, S, GRP)]
G_SCALE = 1.0 / np.sqrt(GDH)
L_SCALE = 1.0 / np.sqrt(LDH)

_CACHE = {}


def _win_start(q, phase):
    if phase == 0:
        return 10 * (q // 10)
    if q < 5:
        return None
    return 10 * ((q - 5) // 10) + 5


MASK_M = 512.0   # exact in bf16; exp arg gets -MASK_M*L_SCALE ~ -45 off-block
# variant index per (g, p): A-full, A-tail, B-0, B-full, B-4
MASK_VARIANT = {}
for _g in range(5):
    MASK_VARIANT[(_g, 0)] = 0 if _g < 4 else 1
    MASK_VARIANT[(_g, 1)] = 2 if _g == 0 else (3 if _g < 4 else 4)
_VARIANT_REP = {0: (0, 0), 1: (4, 0), 2: (0, 1), 3: (1, 1), 4: (4, 1)}


def _build_mask_uv():
    """Rank-16 factors of the block-diag mask: mask = (u.T @ v) with
    u[w,k]=M on window w's keys, v[w,q]=1 on its queries (per variant).
    exp(scores + u.T@v - M) realizes the multiplicative mask."""
    u = np.zeros((5, 16, 128), np.float32)
    v = np.zeros((5, 16, 4, GRP), np.float32)
    for var, (g, p) in _VARIANT_REP.items():
        q0, q1 = GROUPS[g]
        k0 = q0 if p == 0 else max(q0 - 5, 0)
        wins = {}
        for q in range(q0, q1):
            st = _win_start(q, p)
            wins.setdefault(st, []).append(q)
        wi = 0
        for st, qs in sorted(wins.items(), key=lambda t: (t[0] is None, t[0])):
            if st is None:
                u[var, 15, 0] = MASK_M          # dummy key; zeroed post-norm
                for q in qs:
                    v[var, 15, :, q - q0] = 1.0
                continue
            for kk in range(st, min(st + W, S)):
                u[var, wi, kk - k0] = MASK_M
            for q in qs:
                v[var, wi, :, q - q0] = 1.0
            wi += 1
    return u, v.reshape(5, 16, 4 * GRP)


def _key_range(g, p):
    q0, q1 = GROUPS[g]
    if p == 0:
        return q0, q1
    return max(q0 - 5, 0), min(q1 + 5, S)


def _build_nc(reps=1):
    import concourse.bass as bass  # noqa: F401
    import concourse.mybir as mybir
    import concourse.tile as tile
    from concourse import bacc

    F32 = mybir.dt.float32
    F32R = mybir.dt.float32r
    BF16 = mybir.dt.bfloat16
    AF = mybir.ActivationFunctionType

    nc = bacc.Bacc("TRN2", target_bir_lowering=False, debug=False,
                   num_devices=NCORES)

    xT = nc.dram_tensor("xT", [BPC, D, S], BF16, kind="ExternalInput")
    wnames = ["wq_g", "wk_g", "wv_g", "wq_l", "wk_l", "wv_l", "wo_g", "wo_l"]
    wdr = {n: nc.dram_tensor(n, [D, D], BF16, kind="ExternalInput")
           for n in wnames}
    fwT = nc.dram_tensor("fwT", [2 * D, D], BF16, kind="ExternalInput")
    cst = nc.dram_tensor("cst", [128, 128], F32R, kind="ExternalInput")
    lmask_u = nc.dram_tensor("lmask_u", [5, 16, 128], BF16,
                             kind="ExternalInput")
    lmask_v = nc.dram_tensor("lmask_v", [5, 16, 4 * GRP], BF16,
                             kind="ExternalInput")
    out = nc.dram_tensor("out", [BPC, S, D], F32, kind="ExternalOutput")

    with tile.TileContext(nc) as tc:
        with (
            tc.tile_pool(name="const", bufs=1) as cp,
            tc.tile_pool(name="work", bufs=1) as wp,
            tc.tile_pool(name="pmm", bufs=2, space="PSUM") as pmm,
            tc.tile_pool(name="psc", bufs=2, space="PSUM") as psc,
            tc.tile_pool(name="pav", bufs=2, space="PSUM") as pav,
            tc.tile_pool(name="prep", bufs=2, space="PSUM") as prep,
        ):
            # ---------------- constants (first-use DMA order) ----------
            xt0 = wp.tile([128, 4, S], BF16, tag="xt", bufs=2)
            nc.sync.dma_start(
                xt0[:], xT[0].rearrange("(kc p) t -> p kc t", p=128))
            w_sb = {}
            for n in ["wq_g", "wk_g", "wq_l", "wk_l", "wv_g", "wv_l",
                      "wo_g", "wo_l"]:
                t = cp.tile([128, 4, D], BF16, tag=f"w_{n}")
                nc.sync.dma_start(
                    t[:], wdr[n].rearrange("(kc p) n -> p kc n", p=128))
                w_sb[n] = t
            ones_gr = cp.tile([128, 128], F32R, tag="ones_gr")
            nc.sync.dma_start(ones_gr[:], cst[:, :])
            ones_b = cp.tile([128, 128], BF16, tag="ones_b")
            nc.vector.tensor_copy(ones_b[:], ones_gr[:])
            mu_sb = cp.tile([16, 5, 128], BF16, tag="lmask_u")
            nc.sync.dma_start(mu_sb[:], lmask_u.rearrange("g w k -> w g k"))
            mv_sb = cp.tile([16, 5, 4 * GRP], BF16, tag="lmask_v")
            nc.sync.dma_start(mv_sb[:], lmask_v.rearrange("g w n -> w g n"))
            fw_sb = cp.tile([128, 8, D], BF16, tag="w_fw")
            nc.sync.dma_start(
                fw_sb[:], fwT.rearrange("(kc p) n -> p kc n", p=128))
            zeros20 = cp.tile([128, 20], F32, tag="zeros20")
            nc.vector.memset(zeros20[:], 0.0)
            mbias = cp.tile([128, 1], F32, tag="mbias")
            nc.vector.memset(mbias[:], -MASK_M * L_SCALE)

            def proj_fm(w, xt, tag):
                """Feature-major projection: out[128, 4, S] = w.T-style."""
                r = wp.tile([128, 4, S], BF16, tag=tag, bufs=2)
                for mc in range(4):
                    ps = pmm.tile([128, S], F32, tag="pmm")
                    for kc in range(4):
                        nc.tensor.matmul(
                            ps[:], w[:, kc, mc * 128:(mc + 1) * 128],
                            xt[:, kc, :], start=(kc == 0), stop=(kc == 3))
                    nc.vector.tensor_copy(r[:, mc, :], ps[:])
                return r

            def emit_batch(bi, use_xt0=False):
                if use_xt0:
                    xt = xt0
                else:
                    xt = wp.tile([128, 4, S], BF16, tag="xt", bufs=2)
                    nc.sync.dma_start(
                        xt[:], xT[bi].rearrange("(kc p) t -> p kc t", p=128))

                # ---------- global branch ----------
                qg = proj_fm(w_sb["wq_g"], xt, "qfm")
                kg = proj_fm(w_sb["wk_g"], xt, "kfm")
                # v token-major with per-head ones column: [128, tc, 8, 65]
                vg = wp.tile([128, 4, 8, 65], BF16, tag="vg")
                for tcc in range(4):
                    ps = pmm.tile([128, S], F32, tag="pmm")
                    for kc in range(4):
                        nc.tensor.matmul(
                            ps[:], xt[:, kc, tcc * 128:(tcc + 1) * 128],
                            w_sb["wv_g"][:, kc, :],
                            start=(kc == 0), stop=(kc == 3))
                    nc.scalar.copy(
                        vg[:, tcc, :, 0:64],
                        ps[:].rearrange("p (h e) -> p h e", h=8))
                    nc.vector.tensor_copy(
                        vg[:, tcc, :, 64:65],
                        ones_b[:, 0:8].rearrange("p (h o) -> p h o", h=8))

                ql = proj_fm(w_sb["wq_l"], xt, "qfm")
                kl = proj_fm(w_sb["wk_l"], xt, "kfm")

                gout = wp.tile([128, 4, S], BF16, tag="gout")
                for h in range(GH):
                    th, po = h // 2, 64 * (h % 2)
                    e_tiles = []
                    for kc in range(4):
                        ps_s = psc.tile([128, S], F32, tag="psc")
                        nc.tensor.matmul(
                            ps_s[:],
                            kg[po:po + 64, th, kc * 128:(kc + 1) * 128],
                            qg[po:po + 64, th, :])
                        e = wp.tile([128, S], BF16, tag="gE", bufs=3)
                        nc.scalar.activation(e[:], ps_s[:], AF.Exp,
                                             scale=G_SCALE)
                        e_tiles.append(e)
                    ps_av = pav.tile([128, S], F32, tag="pav")
                    for kc in range(4):
                        nc.tensor.matmul(
                            ps_av[0:65, :], vg[:, kc, h, :],
                            e_tiles[kc][:],
                            start=(kc == 0), stop=(kc == 3))
                    den = wp.tile([1, S], F32R, tag="den")
                    nc.scalar.copy(den[0:1, :], ps_av[64:65, :])
                    ps_rep = prep.tile([64, S], F32, tag="prep")
                    nc.tensor.matmul(ps_rep[:], ones_gr[0:1, 0:64],
                                     den[0:1, :])
                    rg = wp.tile([64, S], F32R, tag="rg")
                    with nc.allow_low_precision(reason="f32r softmax denom"):
                        nc.vector.reciprocal(rg[:], ps_rep[:])
                    nc.vector.tensor_mul(
                        gout[po:po + 64, th, :],
                        ps_av[0:64, :], rg[0:64, :])

                yg = wp.tile([128, 4, S], BF16, tag="yg")
                for ec in range(4):
                    ps = pmm.tile([128, S], F32, tag="pmm")
                    for kc in range(4):
                        nc.tensor.matmul(
                            ps[:], w_sb["wo_g"][:, kc, ec * 128:(ec + 1) * 128],
                            gout[:, kc, :], start=(kc == 0), stop=(kc == 3))
                    nc.vector.tensor_copy(yg[:, ec, :], ps[:])

                # ---------- local branch ----------
                lout = wp.tile([128, 4, S], BF16, tag="lout")
                for g, (q0, q1) in enumerate(GROUPS):
                    nq = q1 - q0
                    en_tiles = {}
                    kr = {}
                    for p in (0, 1):
                        k0, k1 = _key_range(g, p)
                        nk = k1 - k0
                        kr[p] = (k0, k1, nk)
                        # v for this key range, token-major [nk, 512]
                        vl = wp.tile([128, S], BF16, tag=f"vl{p}")
                        ps_v = pmm.tile([128, S], F32, tag="pmm")
                        for kc in range(4):
                            nc.tensor.matmul(
                                ps_v[0:nk, :], xt[:, kc, k0:k1],
                                w_sb["wv_l"][:, kc, :],
                                start=(kc == 0), stop=(kc == 3))
                        nc.vector.tensor_copy(vl[0:nk, :], ps_v[0:nk, :])
                        # scores^T [keys, 4 heads x queries]; the rank-16
                        # mask matmul seeds +M on in-window pairs, exp's
                        # bias of -M turns that into a multiplicative mask
                        var = MASK_VARIANT[(g, p)]
                        ps_ls = psc.tile([128, 4 * GRP], F32, tag="psc")
                        nc.tensor.matmul(
                            ps_ls[0:nk, :], mu_sb[:, var, 0:nk],
                            mv_sb[:, var, :], start=True, stop=False,
                            skip_group_check=True)
                        for h in range(LH):
                            nc.tensor.matmul(
                                ps_ls[0:nk, h * GRP:h * GRP + nq],
                                kl[:, h, k0:k1], ql[:, h, q0:q1],
                                start=False, stop=(h == LH - 1),
                                skip_group_check=True)
                        el = wp.tile([128, 4 * GRP], BF16, tag="el", bufs=2)
                        nc.scalar.activation(
                            el[0:nk, :], ps_ls[0:nk, :], AF.Exp,
                            scale=L_SCALE, bias=mbias[0:nk])
                        ps_den = prep.tile([128, 4 * GRP], F32, tag="prep")
                        nc.tensor.matmul(ps_den[:, :], ones_b[0:nk, :],
                                         el[0:nk, :])
                        rl = wp.tile([128, 4 * GRP], F32R, tag="rl", bufs=2)
                        with nc.allow_low_precision(reason="f32r softmax denom"):
                            nc.vector.reciprocal(rl[0:nk, :], ps_den[0:nk, :])
                        en = wp.tile([128, 4 * GRP], BF16, tag=f"en{p}",
                                     bufs=2)
                        nc.vector.tensor_mul(en[0:nk, :], el[0:nk, :],
                                             rl[0:nk, :])
                        if g == 0 and p == 1:
                            # queries 0..4 have no odd window: zero them
                            nc.vector.tensor_copy(
                                en[0:nk, :].rearrange(
                                    "p (h q) -> p h q", h=4)[:, :, 0:5],
                                zeros20[0:nk, :].rearrange(
                                    "p (h q) -> p h q", h=4))
                        en_tiles[p] = (en, vl)
                    ps_lav = pav.tile([128, 4 * GRP], F32, tag="pav")
                    for h in range(LH):
                        for p in (0, 1):
                            k0, k1, nk = kr[p]
                            en, vl = en_tiles[p]
                            nc.tensor.matmul(
                                ps_lav[:, h * GRP:h * GRP + nq],
                                vl[0:nk, h * 128:(h + 1) * 128],
                                en[0:nk, h * GRP:h * GRP + nq],
                                start=(p == 0), stop=(p == 1))
                    for h in range(LH):
                        nc.scalar.copy(lout[:, h, q0:q1],
                                       ps_lav[:, h * GRP:h * GRP + nq])

                yl = wp.tile([128, 4, S], BF16, tag="yl")
                for ec in range(4):
                    ps = pmm.tile([128, S], F32, tag="pmm")
                    for kc in range(4):
                        nc.tensor.matmul(
                            ps[:], w_sb["wo_l"][:, kc, ec * 128:(ec + 1) * 128],
                            lout[:, kc, :], start=(kc == 0), stop=(kc == 3))
                    nc.vector.tensor_copy(yl[:, ec, :], ps[:])

                # ---------- fusion ----------
                for tcc in range(4):
                    ps = pmm.tile([128, S], F32, tag="pmm")
                    for fc in range(8):
                        src = yg if fc < 4 else yl
                        nc.tensor.matmul(
                            ps[:], src[:, fc % 4, tcc * 128:(tcc + 1) * 128],
                            fw_sb[:, fc, :], start=(fc == 0), stop=(fc == 7))
                    res = wp.tile([128, S], F32, tag="res")
                    nc.scalar.activation(res[:], ps[:], AF.Relu)
                    nc.sync.dma_start(
                        out[bi, tcc * 128:(tcc + 1) * 128, :], res[:])

            if reps == 1:
                for bi in range(BPC):
                    emit_batch(bi, use_xt0=(bi == 0))
            else:
                # xt0 only carries real data on the first trip; use fresh
                # DMAs inside the loop (timing variant, results unused)
                with tc.For_i(0, reps, 1, hint_engines=(
                        mybir.EngineType.PE, mybir.EngineType.Activation,
                        mybir.EngineType.DVE, mybir.EngineType.SP,
                        mybir.EngineType.Pool)):
                    for bi in range(BPC):
                        emit_batch(bi)

    nc.compile()
    return nc


def host_in_maps(x, gw_in, gw_out, lw_in, lw_out, fw):
    """Per-core input maps: batch-sharded x^T + transposed weights (bf16)."""
    bf = ml_dtypes.bfloat16
    x = np.ascontiguousarray(np.asarray(x, np.float32))
    gw_in = np.asarray(gw_in, np.float32)
    lw_in = np.asarray(lw_in, np.float32)
    consts = {
        "wq_g": np.ascontiguousarray(gw_in[0:D].T).astype(bf),
        "wk_g": np.ascontiguousarray(gw_in[D:2 * D].T).astype(bf),
        "wv_g": np.ascontiguousarray(gw_in[2 * D:3 * D].T).astype(bf),
        "wq_l": np.ascontiguousarray(lw_in[0:D].T).astype(bf),
        "wk_l": np.ascontiguousarray(lw_in[D:2 * D].T).astype(bf),
        "wv_l": np.ascontiguousarray(lw_in[2 * D:3 * D].T).astype(bf),
        "wo_g": np.ascontiguousarray(np.asarray(gw_out, np.float32).T).astype(bf),
        "wo_l": np.ascontiguousarray(np.asarray(lw_out, np.float32).T).astype(bf),
        "fwT": np.ascontiguousarray(np.asarray(fw, np.float32).T).astype(bf),
    }

    mu, mv = _build_mask_uv()
    consts["lmask_u"] = mu.astype(bf)
    consts["lmask_v"] = mv.astype(bf)
    consts["cst"] = np.ones((128, 128), np.float32)

    in_maps = []
    for c in range(NCORES):
        xb = np.ascontiguousarray(
            x[c * BPC:(c + 1) * BPC].transpose(0, 2, 1)).astype(bf)
        in_maps.append({"xT": xb, **consts})
    return in_maps


def kernel(x, gw_in, gb_in, gw_out, gb_out, lw_in, lb_in, lw_out, lb_out,
           fw, fb):
    import sys
    if '/opt/trn_rl_repo' not in sys.path:
        sys.path.insert(0, '/opt/trn_rl_repo')
    from concourse.bass_utils import run_bass_kernel_spmd

    in_maps = host_in_maps(x, gw_in, gw_out, lw_in, lw_out, fw)
    if "nc" not in _CACHE:
        _CACHE["nc"] = _build_nc()
    nc = _CACHE["nc"]
    res = run_bass_kernel_spmd(nc, in_maps, core_ids=list(range(NCORES)))
    return np.concatenate([r["out"] for r in res.results], axis=0)
